# revision 26
# baseline (speedup 1.0000x reference)
"""GQA attention (B=2, S=2048, D=2048, H=32, G=8, hd=64) on 8 TRN2 cores.

Sharding: 2 batch groups x 4 TP ranks, NO collectives. Core c: batch
b=c//4, rank r=c%4. Each rank owns 2 KV groups (8 Q heads), computes a
PARTIAL output projection over its 512 local head-features, and the host
sums the 4 rank partials per batch.

Layout strategy (all transposes done on host):
  - x^T resident in SBUF; projections produce Q^T/K^T [feat, tok] and
    V [tok, feat] directly, so scores S^T [k, q] come out transpose-free
    and P^T blocks feed the PV matmul as the stationary operand.
  - scores: grp A on PE rows 0:64 / grp B on rows 64:128 via
    tile_position packing -> the two matmuls run concurrently.
  - softmax denominator: the PV stationary is [V_g (64) | ones (64)], so
    PSUM rows 64:128 accumulate sum_k(exp) replicated across 64
    partitions for free. Drain = copy denom to base-0 + fast approx
    reciprocal + fused multiply (no Ln/Exp table swaps, no broadcasts).
  - PSUM: one [128,2048] score tile holds TWO kb blocks (A|B|A|B), so
    EXP runs as one [128,2048] activation per kb-pair (less per-instr
    overhead); o accumulators single-buffered pair; remaining 2 banks
    shared by projections and the dripped output projection.
  - attention runs per 512-token group (qg); the output projection of
    group g is interleaved ("dripped") into group g+1's attention so it
    hides in the PE slack of the scalar-bound exp pipeline.
"""

import sys

sys.path.insert(0, "/opt/trn_rl_repo")

import numpy as np
import ml_dtypes

import concourse.bass as bass
import concourse.tile as tile
from concourse import bacc, mybir
from concourse.bass_utils import run_bass_kernel_spmd

BF16 = ml_dtypes.bfloat16
B, S, D = 2, 2048, 2048
H, G, HD = 32, 8, 64
N_CORES = 8
TP = 4
QF = 512   # q features per rank
DC = D // 128  # 16 dim chunks
QT = 1024  # q tokens per Q-projection tile

_CACHE = {}


def _build():
    f32 = mybir.dt.float32
    bf16 = mybir.dt.bfloat16
    nc = bacc.Bacc("TRN2", target_bir_lowering=False, debug=False, num_devices=N_CORES)

    xt = nc.dram_tensor("xt", [128, DC, S], bf16, kind="ExternalInput").ap()
    wqt = nc.dram_tensor("wqt", [128, DC, QF], bf16, kind="ExternalInput").ap()
    wkt = nc.dram_tensor("wkt", [128, DC, 128], bf16, kind="ExternalInput").ap()
    wvt = nc.dram_tensor("wvt", [128, DC, 128], bf16, kind="ExternalInput").ap()
    cosr = nc.dram_tensor("cosr", [128, S], bf16, kind="ExternalInput").ap()
    sinr = nc.dram_tensor("sinr", [128, S], bf16, kind="ExternalInput").ap()
    wot = nc.dram_tensor("wot", [128, 4, D], bf16, kind="ExternalInput").ap()
    out = nc.dram_tensor("out", [S, D], f32, kind="ExternalOutput").ap()

    Exp = mybir.ActivationFunctionType.Exp
    swap_mask = [i ^ 1 for i in range(32)]
    scale = float(1.0 / np.sqrt(HD))

    from contextlib import ExitStack
    with tile.TileContext(nc) as tc, ExitStack() as ctx:
        consts = ctx.enter_context(tc.tile_pool(name="consts", bufs=1))
        qk = ctx.enter_context(tc.tile_pool(name="qk", bufs=1))
        io = ctx.enter_context(tc.tile_pool(name="io", bufs=2))
        work = ctx.enter_context(tc.tile_pool(name="work", bufs=3))
        dr = ctx.enter_context(tc.tile_pool(name="dr", bufs=4))
        ost = ctx.enter_context(tc.tile_pool(name="ost", bufs=3))
        # PSUM (8 banks): sp 2x[128,1024]=4, o pairs bufs=3 x [128,512]=3,
        # pp 1 bank for dripped output-projection units
        psum = ctx.enter_context(tc.tile_pool(name="psum", bufs=2, space="PSUM"))
        opsum = ctx.enter_context(tc.tile_pool(name="opsum", bufs=3, space="PSUM"))
        pp = ctx.enter_context(tc.tile_pool(name="pp", bufs=1, space="PSUM"))

        # ---- load inputs (x^T split in 4 tiles so K-proj starts early)
        xt_sb = [
            qk.tile([128, 4, S], bf16, tag=f"xt{i}", name=f"xt_sb{i}")
            for i in range(4)
        ]
        # issue order follows first use: K proj needs wkt + early x chunks
        wkt_sb = consts.tile([128, DC, 128], bf16, tag="wkt")
        nc.sync.dma_start(out=wkt_sb[:], in_=wkt[:])
        for i in range(2):
            nc.sync.dma_start(out=xt_sb[i][:], in_=xt[:, 4 * i:4 * i + 4, :])
        wvt_sb = consts.tile([128, DC, 128], bf16, tag="wvt")
        nc.sync.dma_start(out=wvt_sb[:], in_=wvt[:])
        for i in range(2, 4):
            nc.sync.dma_start(out=xt_sb[i][:], in_=xt[:, 4 * i:4 * i + 4, :])
        cos_sb = consts.tile([128, S], bf16, tag="cos")
        nc.sync.dma_start(out=cos_sb[:], in_=cosr[:])
        sin_sb = consts.tile([128, S], bf16, tag="sin")
        nc.sync.dma_start(out=sin_sb[:], in_=sinr[:])
        wqt_sb = consts.tile([128, DC, QF], bf16, tag="wqt")
        nc.sync.dma_start(out=wqt_sb[:], in_=wqt[:])
        wot_sb = consts.tile([128, 4, D], bf16, tag="wot")
        nc.sync.dma_start(out=wot_sb[:], in_=wot[:])

        def xs(c):  # x^T chunk c
            return xt_sb[c // 4][:, c % 4, :]

        # ---- K^T projection, c-outer so (1) it starts once the first x^T
        # quarter lands and (2) the stationary is loaded once per c (4 nt
        # accumulate in parallel PSUM tiles)
        kt_sb = qk.tile([128, S], bf16, tag="kt")
        kps = [opsum.tile([128, 512], f32, tag="o", name=f"kps{i}")
               for i in range(3)]
        kps.append(pp.tile([128, 512], f32, tag="pp", name="kps3"))
        for c in range(DC):
            for nt in range(4):
                nc.tensor.matmul(
                    kps[nt], lhsT=wkt_sb[:, c, :],
                    rhs=xs(c)[:, nt * 512:(nt + 1) * 512],
                    start=(c == 0), stop=(c == DC - 1),
                )
        for nt in range(4):
            nc.vector.tensor_copy(kt_sb[:, nt * 512:(nt + 1) * 512], kps[nt])
        for hc in range(2):
            hsl = slice(hc * QT, (hc + 1) * QT)
            sw = io.tile([128, QT], bf16, tag="rsw")
            nc.vector.stream_shuffle(sw, kt_sb[:, hsl], swap_mask)
            nc.vector.tensor_mul(sw, sw, sin_sb[:, hsl])
            tmp = io.tile([128, QT], bf16, tag="rtmp")
            nc.vector.tensor_mul(tmp, kt_sb[:, hsl], cos_sb[:, hsl])
            nc.vector.tensor_add(kt_sb[:, hsl], sw, tmp)

        # ---- V projection into PV-stationary layout:
        # vtile[:, tb, 0:64]=V_A, 64:128=ones, 128:192=V_B, 192:256=ones
        vtile = qk.tile([128, DC, 256], bf16, tag="vtile")
        nc.vector.memset(vtile[:, :, 64:128], 1.0)
        nc.vector.memset(vtile[:, :, 192:256], 1.0)
        for tb in range(DC):
            ps = opsum.tile([128, 512], f32, tag="o")
            for c in range(DC):
                nc.tensor.matmul(
                    ps[:, 0:128],
                    lhsT=xs(c)[:, tb * 128:(tb + 1) * 128],
                    rhs=wvt_sb[:, c, :],
                    start=(c == 0), stop=(c == DC - 1),
                )
            nc.vector.tensor_copy(vtile[:, tb, 0:64], ps[:, 0:64])
            nc.vector.tensor_copy(vtile[:, tb, 128:192], ps[:, 64:128])

        # ---- Q^T projection (c-outer: stationary loaded once per (j,c),
        # both 512-token halves of the qtile accumulate in parallel) + RoPE
        qt_sb = [
            qk.tile([128, 4, QT], bf16, tag=f"qt{q}", name=f"qt_sb{q}")
            for q in range(2)
        ]
        ot_sb = [
            qk.tile([128, 4, QT], bf16, tag=f"ot{q}", name=f"ot_sb{q}")
            for q in range(2)
        ]
        for q in range(2):
            for j in range(4):
                qps = [opsum.tile([128, 512], f32, tag="o", name=f"qps{i}")
                       for i in range(2)]
                for c in range(DC):
                    for nt in range(2):
                        tsl = slice(q * QT + nt * 512, q * QT + (nt + 1) * 512)
                        nc.tensor.matmul(
                            qps[nt], lhsT=wqt_sb[:, c, j * 128:(j + 1) * 128],
                            rhs=xs(c)[:, tsl],
                            start=(c == 0), stop=(c == DC - 1),
                        )
                for nt in range(2):
                    nc.vector.tensor_copy(
                        qt_sb[q][:, j, nt * 512:(nt + 1) * 512], qps[nt])
                qsl = slice(q * QT, (q + 1) * QT)
                t = qt_sb[q][:, j, :]
                sw = io.tile([128, QT], bf16, tag="rsw")
                nc.vector.stream_shuffle(sw, t, swap_mask)
                nc.vector.tensor_mul(sw, sw, sin_sb[:, qsl])
                tmp = io.tile([128, QT], bf16, tag="rtmp")
                nc.vector.tensor_mul(tmp, t, cos_sb[:, qsl])
                nc.vector.tensor_add(t, sw, tmp)

        # ---- attention per 512-token group + dripped output projection
        def oproj_unit(qg, tb, od, pool, use_scalar=False):
            """One output-projection unit: 4 matmuls + drain + DMA."""
            q = qg // 2
            gtb = qg * 4 + tb
            ps = pool.tile([128, 512], f32, tag="pp" if pool is pp else "o")
            for ic in range(4):
                nc.tensor.matmul(
                    ps,
                    lhsT=ot_sb[q][:, ic, (qg % 2) * 512 + tb * 128:
                                  (qg % 2) * 512 + (tb + 1) * 128],
                    rhs=wot_sb[:, ic, od * 512:(od + 1) * 512],
                    start=(ic == 0), stop=(ic == 3),
                )
            osb = ost.tile([128, 512], f32, tag="osb")
            if use_scalar:
                nc.scalar.activation(
                    osb, ps, mybir.ActivationFunctionType.Copy)
            else:
                nc.vector.tensor_copy(osb, ps)
            nc.sync.dma_start(
                out=out[gtb * 128:(gtb + 1) * 128, od * 512:(od + 1) * 512],
                in_=osb)

        pending = []
        for q in range(2):
            for qh in range(2):
                qg = 2 * q + qh
                qsl = slice(qh * 512, (qh + 1) * 512)
                for j in range(4):
                    oA = opsum.tile([128, 512], f32, tag="o", name="oA")
                    oB = opsum.tile([128, 512], f32, tag="o", name="oB")
                    for kb in range(DC):
                        ksl = slice(kb * 128, (kb + 1) * 128)
                        sp = psum.tile([128, 1024], f32, tag="sp")
                        nc.tensor.matmul(
                            sp[:, 0:512], lhsT=kt_sb[0:64, ksl],
                            rhs=qt_sb[q][0:64, j, qsl],
                            start=True, stop=True, tile_position=(0, 0),
                        )
                        nc.tensor.matmul(
                            sp[:, 512:1024], lhsT=kt_sb[64:128, ksl],
                            rhs=qt_sb[q][64:128, j, qsl],
                            start=True, stop=True, tile_position=(64, 0),
                        )
                        p = work.tile([128, 1024], bf16, tag="p")
                        nc.scalar.activation(p, sp, Exp, scale=scale)
                        nc.tensor.matmul(
                            oA, lhsT=vtile[:, kb, 0:128], rhs=p[:, 0:512],
                            start=(kb == 0), stop=(kb == DC - 1),
                        )
                        nc.tensor.matmul(
                            oB, lhsT=vtile[:, kb, 128:256], rhs=p[:, 512:1024],
                            start=(kb == 0), stop=(kb == DC - 1),
                        )
                        # drip one output-projection unit per 4 kb — fits in
                        # the PE slack of the scalar-bound exp pipeline
                        if pending and kb % 4 == 3:
                            pending.pop(0)(pp)
                    # drain: copy denom rows to base 0, fast reciprocal
                    # (base-aligned SBUF), multiply from PSUM data rows
                    for grp, o in ((0, oA), (1, oB)):
                        dcp = dr.tile([64, 512], f32, tag="dcp")
                        nc.vector.tensor_copy(dcp, o[64:128, :])
                        rec = dr.tile([64, 512], f32, tag="rec")
                        nc.vector.reciprocal_approx_fast(out=rec, in_=dcp)
                        nc.vector.tensor_mul(
                            ot_sb[q][64 * grp:64 * grp + 64, j, qsl],
                            o[0:64, :], rec)
                for tb in range(4):
                    for od in range(4):
                        pending.append(
                            (lambda pool, use_scalar=False, qg=qg, tb=tb,
                             od=od:
                             oproj_unit(qg, tb, od, pool, use_scalar)))
        # flush the remaining units, alternating pools so the drain copies
        # never serialize the accumulation; the scalar engine is idle at
        # the tail, so give it half the drain copies
        i = 0
        while pending:
            fn = pending.pop(0)
            fn(pp if i % 2 == 0 else opsum, use_scalar=(i % 2 == 1))
            i += 1

    nc.compile()
    return nc


def _prep_inputs(x, freqs_cos, freqs_sin, wqkv, wo):
    """Build per-core input maps (host-side shard + transpose + bf16 cast)."""
    ins = []
    wo_t = np.ascontiguousarray(wo.T)  # [in feat, out feat]
    cos_h = np.empty((128, S), np.float32)
    sin_h = np.empty((128, S), np.float32)
    cs = freqs_cos[:, 0, :]  # [S, 64]
    sn = freqs_sin[:, 0, :]
    for p in range(128):
        cos_h[p] = cs[:, p % 64]
        sin_h[p] = sn[:, p % 64] * (-1.0 if p % 2 == 0 else 1.0)
    cos_h = cos_h.astype(BF16)
    sin_h = sin_h.astype(BF16)

    for core in range(N_CORES):
        b, r = divmod(core, TP)
        xt_h = np.ascontiguousarray(
            x[b].T.reshape(DC, 128, S).transpose(1, 0, 2)).astype(BF16)
        # Q rows, permuted: j-tile j = [head 8r+j | head 8r+4+j]
        rows = []
        for j in range(4):
            for h in (8 * r + j, 8 * r + 4 + j):
                rows.extend(range(h * HD, (h + 1) * HD))
        wq_sel = wqkv[rows, :]  # [512, D]
        wqt_h = np.ascontiguousarray(
            wq_sel.T.reshape(DC, 128, QF).transpose(1, 0, 2)).astype(BF16)
        krows = []
        for g in (2 * r, 2 * r + 1):
            krows.extend(range(H * HD + g * HD, H * HD + (g + 1) * HD))
        wk_sel = wqkv[krows, :]  # [128, D]
        wkt_h = np.ascontiguousarray(
            wk_sel.T.reshape(DC, 128, 128).transpose(1, 0, 2)).astype(BF16)
        vrows = []
        for g in (2 * r, 2 * r + 1):
            vrows.extend(range((H + G) * HD + g * HD, (H + G) * HD + (g + 1) * HD))
        wv_sel = wqkv[vrows, :]  # [128, D]; cols 0:64=V_A feats, 64:128=V_B
        wvt_h = np.ascontiguousarray(
            wv_sel.T.reshape(DC, 128, 128).transpose(1, 0, 2)).astype(BF16)
        # wot: local head-feature rows, chunk ic=j: [head 8r+j | head 8r+4+j]
        perm = np.empty(4 * 128, np.int64)
        for j in range(4):
            for p in range(128):
                Hh = 8 * r + j + (4 if p >= 64 else 0)
                perm[j * 128 + p] = 64 * Hh + (p % 64)
        wot_h = np.ascontiguousarray(
            wo_t[perm, :].reshape(4, 128, D).transpose(1, 0, 2)).astype(BF16)
        ins.append({
            "xt": xt_h, "wqt": wqt_h, "wkt": wkt_h, "wvt": wvt_h,
            "cosr": cos_h, "sinr": sin_h, "wot": wot_h,
        })
    return ins


TRACE = False


def kernel(x, freqs_cos, freqs_sin, wqkv, wo):
    if "nc" not in _CACHE:
        _CACHE["nc"] = _build()
    nc = _CACHE["nc"]
    ins = _prep_inputs(x, freqs_cos, freqs_sin, wqkv, wo)
    res = run_bass_kernel_spmd(nc, ins, list(range(N_CORES)), trace=TRACE)
    _CACHE["res"] = res
    out = np.empty((B, S, D), np.float32)
    for b in range(B):
        acc = res.results[TP * b]["out"].astype(np.float32)
        for r in range(1, TP):
            acc = acc + res.results[TP * b + r]["out"]
        out[b] = acc
    return out


if __name__ == "__main__":
    rng = np.random.default_rng(0)
    x = rng.normal(size=(B, S, D)).astype(np.float32)
    fc = rng.random(size=(S, 1, HD)).astype(np.float32)
    fs = rng.random(size=(S, 1, HD)).astype(np.float32)
    wq = rng.normal(size=(3072, D)).astype(np.float32) * 0.02
    wo = rng.normal(size=(D, D)).astype(np.float32) * 0.02
    o = kernel(x, fc, fs, wq, wo)
    print(o.shape, o.dtype)


# revision 28
# speedup vs baseline: 1.0011x; 1.0011x over previous
"""GQA attention (B=2, S=2048, D=2048, H=32, G=8, hd=64) on 8 TRN2 cores.

Sharding: 2 batch groups x 4 TP ranks, NO collectives. Core c: batch
b=c//4, rank r=c%4. Each rank owns 2 KV groups (8 Q heads), computes a
PARTIAL output projection over its 512 local head-features, and the host
sums the 4 rank partials per batch.

Layout strategy (all transposes done on host):
  - x^T resident in SBUF; projections produce Q^T/K^T [feat, tok] and
    V [tok, feat] directly, so scores S^T [k, q] come out transpose-free
    and P^T blocks feed the PV matmul as the stationary operand.
  - scores: grp A on PE rows 0:64 / grp B on rows 64:128 via
    tile_position packing -> the two matmuls run concurrently.
  - softmax denominator: the PV stationary is [V_g (64) | ones (64)], so
    PSUM rows 64:128 accumulate sum_k(exp) replicated across 64
    partitions for free. Drain = copy denom to base-0 + fast approx
    reciprocal + fused multiply (no Ln/Exp table swaps, no broadcasts).
  - PSUM: one [128,2048] score tile holds TWO kb blocks (A|B|A|B), so
    EXP runs as one [128,2048] activation per kb-pair (less per-instr
    overhead); o accumulators single-buffered pair; remaining 2 banks
    shared by projections and the dripped output projection.
  - attention runs per 512-token group (qg); the output projection of
    group g is interleaved ("dripped") into group g+1's attention so it
    hides in the PE slack of the scalar-bound exp pipeline.
"""

import sys

sys.path.insert(0, "/opt/trn_rl_repo")

import numpy as np
import ml_dtypes

import concourse.bass as bass
import concourse.tile as tile
from concourse import bacc, mybir
from concourse.bass_utils import run_bass_kernel_spmd

BF16 = ml_dtypes.bfloat16
B, S, D = 2, 2048, 2048
H, G, HD = 32, 8, 64
N_CORES = 8
TP = 4
QF = 512   # q features per rank
DC = D // 128  # 16 dim chunks
QT = 1024  # q tokens per Q-projection tile

_CACHE = {}


def _build():
    f32 = mybir.dt.float32
    bf16 = mybir.dt.bfloat16
    nc = bacc.Bacc("TRN2", target_bir_lowering=False, debug=False, num_devices=N_CORES)

    xt = nc.dram_tensor("xt", [128, DC, S], bf16, kind="ExternalInput").ap()
    wqt = nc.dram_tensor("wqt", [128, DC, QF], bf16, kind="ExternalInput").ap()
    wkt = nc.dram_tensor("wkt", [128, DC, 128], bf16, kind="ExternalInput").ap()
    wvt = nc.dram_tensor("wvt", [128, DC, 128], bf16, kind="ExternalInput").ap()
    cosr = nc.dram_tensor("cosr", [128, S], bf16, kind="ExternalInput").ap()
    sinr = nc.dram_tensor("sinr", [128, S], bf16, kind="ExternalInput").ap()
    wot = nc.dram_tensor("wot", [128, 4, D], bf16, kind="ExternalInput").ap()
    out = nc.dram_tensor("out", [S, D], f32, kind="ExternalOutput").ap()

    Exp = mybir.ActivationFunctionType.Exp
    swap_mask = [i ^ 1 for i in range(32)]
    scale = float(1.0 / np.sqrt(HD))

    from contextlib import ExitStack
    with tile.TileContext(nc) as tc, ExitStack() as ctx:
        consts = ctx.enter_context(tc.tile_pool(name="consts", bufs=1))
        qk = ctx.enter_context(tc.tile_pool(name="qk", bufs=1))
        io = ctx.enter_context(tc.tile_pool(name="io", bufs=2))
        work = ctx.enter_context(tc.tile_pool(name="work", bufs=3))
        dr = ctx.enter_context(tc.tile_pool(name="dr", bufs=4))
        ost = ctx.enter_context(tc.tile_pool(name="ost", bufs=3))
        # PSUM (8 banks): sp 2x[128,1024]=4, o pairs bufs=3 x [128,512]=3,
        # pp 1 bank for dripped output-projection units
        psum = ctx.enter_context(tc.tile_pool(name="psum", bufs=2, space="PSUM"))
        opsum = ctx.enter_context(tc.tile_pool(name="opsum", bufs=3, space="PSUM"))
        pp = ctx.enter_context(tc.tile_pool(name="pp", bufs=1, space="PSUM"))

        # ---- load inputs (x^T split in 4 tiles so K-proj starts early)
        xt_sb = [
            qk.tile([128, 4, S], bf16, tag=f"xt{i}", name=f"xt_sb{i}")
            for i in range(4)
        ]
        wkt_sb = consts.tile([128, DC, 128], bf16, tag="wkt")
        nc.sync.dma_start(out=wkt_sb[:], in_=wkt[:])
        for i in range(4):
            nc.sync.dma_start(out=xt_sb[i][:], in_=xt[:, 4 * i:4 * i + 4, :])
        wvt_sb = consts.tile([128, DC, 128], bf16, tag="wvt")
        nc.sync.dma_start(out=wvt_sb[:], in_=wvt[:])
        cos_sb = consts.tile([128, S], bf16, tag="cos")
        nc.sync.dma_start(out=cos_sb[:], in_=cosr[:])
        sin_sb = consts.tile([128, S], bf16, tag="sin")
        nc.sync.dma_start(out=sin_sb[:], in_=sinr[:])
        wqt_sb = consts.tile([128, DC, QF], bf16, tag="wqt")
        nc.sync.dma_start(out=wqt_sb[:], in_=wqt[:])
        wot_sb = consts.tile([128, 4, D], bf16, tag="wot")
        nc.sync.dma_start(out=wot_sb[:], in_=wot[:])

        def xs(c):  # x^T chunk c
            return xt_sb[c // 4][:, c % 4, :]

        # ---- K^T projection, c-outer so (1) it starts once the first x^T
        # quarter lands and (2) the stationary is loaded once per c (4 nt
        # accumulate in parallel PSUM tiles)
        kt_sb = qk.tile([128, S], bf16, tag="kt")
        kps = [opsum.tile([128, 512], f32, tag="o", name=f"kps{i}")
               for i in range(3)]
        kps.append(pp.tile([128, 512], f32, tag="pp", name="kps3"))
        for c in range(DC):
            for nt in range(4):
                nc.tensor.matmul(
                    kps[nt], lhsT=wkt_sb[:, c, :],
                    rhs=xs(c)[:, nt * 512:(nt + 1) * 512],
                    start=(c == 0), stop=(c == DC - 1),
                )
        for nt in range(4):
            nc.vector.tensor_copy(kt_sb[:, nt * 512:(nt + 1) * 512], kps[nt])
        for hc in range(2):
            hsl = slice(hc * QT, (hc + 1) * QT)
            sw = io.tile([128, QT], bf16, tag="rsw")
            nc.vector.stream_shuffle(sw, kt_sb[:, hsl], swap_mask)
            nc.vector.tensor_mul(sw, sw, sin_sb[:, hsl])
            tmp = io.tile([128, QT], bf16, tag="rtmp")
            nc.vector.tensor_mul(tmp, kt_sb[:, hsl], cos_sb[:, hsl])
            nc.vector.tensor_add(kt_sb[:, hsl], sw, tmp)

        # ---- V projection into PV-stationary layout:
        # vtile[:, tb, 0:64]=V_A, 64:128=ones, 128:192=V_B, 192:256=ones
        vtile = qk.tile([128, DC, 256], bf16, tag="vtile")
        nc.vector.memset(vtile[:, :, 64:128], 1.0)
        nc.vector.memset(vtile[:, :, 192:256], 1.0)
        for tb in range(DC):
            ps = opsum.tile([128, 512], f32, tag="o")
            for c in range(DC):
                nc.tensor.matmul(
                    ps[:, 0:128],
                    lhsT=xs(c)[:, tb * 128:(tb + 1) * 128],
                    rhs=wvt_sb[:, c, :],
                    start=(c == 0), stop=(c == DC - 1),
                )
            nc.vector.tensor_copy(vtile[:, tb, 0:64], ps[:, 0:64])
            nc.vector.tensor_copy(vtile[:, tb, 128:192], ps[:, 64:128])

        # ---- Q^T projection (c-outer: stationary loaded once per (j,c),
        # both 512-token halves of the qtile accumulate in parallel) + RoPE
        qt_sb = [
            qk.tile([128, 4, QT], bf16, tag=f"qt{q}", name=f"qt_sb{q}")
            for q in range(2)
        ]
        ot_sb = [
            qk.tile([128, 4, QT], bf16, tag=f"ot{q}", name=f"ot_sb{q}")
            for q in range(2)
        ]
        for q in range(2):
            for j in range(4):
                qps = [opsum.tile([128, 512], f32, tag="o", name=f"qps{i}")
                       for i in range(2)]
                for c in range(DC):
                    for nt in range(2):
                        tsl = slice(q * QT + nt * 512, q * QT + (nt + 1) * 512)
                        nc.tensor.matmul(
                            qps[nt], lhsT=wqt_sb[:, c, j * 128:(j + 1) * 128],
                            rhs=xs(c)[:, tsl],
                            start=(c == 0), stop=(c == DC - 1),
                        )
                for nt in range(2):
                    nc.vector.tensor_copy(
                        qt_sb[q][:, j, nt * 512:(nt + 1) * 512], qps[nt])
                qsl = slice(q * QT, (q + 1) * QT)
                t = qt_sb[q][:, j, :]
                sw = io.tile([128, QT], bf16, tag="rsw")
                nc.vector.stream_shuffle(sw, t, swap_mask)
                nc.vector.tensor_mul(sw, sw, sin_sb[:, qsl])
                tmp = io.tile([128, QT], bf16, tag="rtmp")
                nc.vector.tensor_mul(tmp, t, cos_sb[:, qsl])
                nc.vector.tensor_add(t, sw, tmp)

        # ---- attention per 512-token group + dripped output projection
        def oproj_unit(qg, tb, od, pool, use_scalar=False):
            """One output-projection unit: 4 matmuls + drain + DMA."""
            q = qg // 2
            gtb = qg * 4 + tb
            ps = pool.tile([128, 512], f32, tag="pp" if pool is pp else "o")
            for ic in range(4):
                nc.tensor.matmul(
                    ps,
                    lhsT=ot_sb[q][:, ic, (qg % 2) * 512 + tb * 128:
                                  (qg % 2) * 512 + (tb + 1) * 128],
                    rhs=wot_sb[:, ic, od * 512:(od + 1) * 512],
                    start=(ic == 0), stop=(ic == 3),
                )
            osb = ost.tile([128, 512], f32, tag="osb")
            if use_scalar:
                nc.scalar.activation(
                    osb, ps, mybir.ActivationFunctionType.Copy)
            else:
                nc.vector.tensor_copy(osb, ps)
            nc.sync.dma_start(
                out=out[gtb * 128:(gtb + 1) * 128, od * 512:(od + 1) * 512],
                in_=osb)

        pending = []
        for q in range(2):
            for qh in range(2):
                qg = 2 * q + qh
                qsl = slice(qh * 512, (qh + 1) * 512)
                for j in range(4):
                    oA = opsum.tile([128, 512], f32, tag="o", name="oA")
                    oB = opsum.tile([128, 512], f32, tag="o", name="oB")
                    for kb in range(DC):
                        ksl = slice(kb * 128, (kb + 1) * 128)
                        sp = psum.tile([128, 1024], f32, tag="sp")
                        nc.tensor.matmul(
                            sp[:, 0:512], lhsT=kt_sb[0:64, ksl],
                            rhs=qt_sb[q][0:64, j, qsl],
                            start=True, stop=True, tile_position=(0, 0),
                        )
                        nc.tensor.matmul(
                            sp[:, 512:1024], lhsT=kt_sb[64:128, ksl],
                            rhs=qt_sb[q][64:128, j, qsl],
                            start=True, stop=True, tile_position=(64, 0),
                        )
                        p = work.tile([128, 1024], bf16, tag="p")
                        nc.scalar.activation(p, sp, Exp, scale=scale)
                        nc.tensor.matmul(
                            oA, lhsT=vtile[:, kb, 0:128], rhs=p[:, 0:512],
                            start=(kb == 0), stop=(kb == DC - 1),
                        )
                        nc.tensor.matmul(
                            oB, lhsT=vtile[:, kb, 128:256], rhs=p[:, 512:1024],
                            start=(kb == 0), stop=(kb == DC - 1),
                        )
                        # drip one output-projection unit per 4 kb — fits in
                        # the PE slack of the scalar-bound exp pipeline
                        if pending and kb % 4 == 3:
                            pending.pop(0)(pp)
                    # drain: copy denom rows to base 0, fast reciprocal
                    # (base-aligned SBUF), multiply from PSUM data rows
                    for grp, o in ((0, oA), (1, oB)):
                        dcp = dr.tile([64, 512], f32, tag="dcp")
                        nc.vector.tensor_copy(dcp, o[64:128, :])
                        rec = dr.tile([64, 512], f32, tag="rec")
                        nc.vector.reciprocal_approx_fast(out=rec, in_=dcp)
                        nc.vector.tensor_mul(
                            ot_sb[q][64 * grp:64 * grp + 64, j, qsl],
                            o[0:64, :], rec)
                for tb in range(4):
                    for od in range(4):
                        pending.append(
                            (lambda pool, use_scalar=False, qg=qg, tb=tb,
                             od=od:
                             oproj_unit(qg, tb, od, pool, use_scalar)))
        # flush the remaining units, alternating pools so the drain copies
        # never serialize the accumulation
        i = 0
        while pending:
            pending.pop(0)(pp if i % 2 == 0 else opsum)
            i += 1

    nc.compile()
    return nc


def _prep_inputs(x, freqs_cos, freqs_sin, wqkv, wo):
    """Build per-core input maps (host-side shard + transpose + bf16 cast)."""
    ins = []
    wo_t = np.ascontiguousarray(wo.T)  # [in feat, out feat]
    cos_h = np.empty((128, S), np.float32)
    sin_h = np.empty((128, S), np.float32)
    cs = freqs_cos[:, 0, :]  # [S, 64]
    sn = freqs_sin[:, 0, :]
    for p in range(128):
        cos_h[p] = cs[:, p % 64]
        sin_h[p] = sn[:, p % 64] * (-1.0 if p % 2 == 0 else 1.0)
    cos_h = cos_h.astype(BF16)
    sin_h = sin_h.astype(BF16)

    for core in range(N_CORES):
        b, r = divmod(core, TP)
        xt_h = np.ascontiguousarray(
            x[b].T.reshape(DC, 128, S).transpose(1, 0, 2)).astype(BF16)
        # Q rows, permuted: j-tile j = [head 8r+j | head 8r+4+j]
        rows = []
        for j in range(4):
            for h in (8 * r + j, 8 * r + 4 + j):
                rows.extend(range(h * HD, (h + 1) * HD))
        wq_sel = wqkv[rows, :]  # [512, D]
        wqt_h = np.ascontiguousarray(
            wq_sel.T.reshape(DC, 128, QF).transpose(1, 0, 2)).astype(BF16)
        krows = []
        for g in (2 * r, 2 * r + 1):
            krows.extend(range(H * HD + g * HD, H * HD + (g + 1) * HD))
        wk_sel = wqkv[krows, :]  # [128, D]
        wkt_h = np.ascontiguousarray(
            wk_sel.T.reshape(DC, 128, 128).transpose(1, 0, 2)).astype(BF16)
        vrows = []
        for g in (2 * r, 2 * r + 1):
            vrows.extend(range((H + G) * HD + g * HD, (H + G) * HD + (g + 1) * HD))
        wv_sel = wqkv[vrows, :]  # [128, D]; cols 0:64=V_A feats, 64:128=V_B
        wvt_h = np.ascontiguousarray(
            wv_sel.T.reshape(DC, 128, 128).transpose(1, 0, 2)).astype(BF16)
        # wot: local head-feature rows, chunk ic=j: [head 8r+j | head 8r+4+j]
        perm = np.empty(4 * 128, np.int64)
        for j in range(4):
            for p in range(128):
                Hh = 8 * r + j + (4 if p >= 64 else 0)
                perm[j * 128 + p] = 64 * Hh + (p % 64)
        wot_h = np.ascontiguousarray(
            wo_t[perm, :].reshape(4, 128, D).transpose(1, 0, 2)).astype(BF16)
        ins.append({
            "xt": xt_h, "wqt": wqt_h, "wkt": wkt_h, "wvt": wvt_h,
            "cosr": cos_h, "sinr": sin_h, "wot": wot_h,
        })
    return ins


TRACE = False


def kernel(x, freqs_cos, freqs_sin, wqkv, wo):
    if "nc" not in _CACHE:
        _CACHE["nc"] = _build()
    nc = _CACHE["nc"]
    ins = _prep_inputs(x, freqs_cos, freqs_sin, wqkv, wo)
    res = run_bass_kernel_spmd(nc, ins, list(range(N_CORES)), trace=TRACE)
    _CACHE["res"] = res
    out = np.empty((B, S, D), np.float32)
    for b in range(B):
        acc = res.results[TP * b]["out"].astype(np.float32)
        for r in range(1, TP):
            acc = acc + res.results[TP * b + r]["out"]
        out[b] = acc
    return out


if __name__ == "__main__":
    rng = np.random.default_rng(0)
    x = rng.normal(size=(B, S, D)).astype(np.float32)
    fc = rng.random(size=(S, 1, HD)).astype(np.float32)
    fs = rng.random(size=(S, 1, HD)).astype(np.float32)
    wq = rng.normal(size=(3072, D)).astype(np.float32) * 0.02
    wo = rng.normal(size=(D, D)).astype(np.float32) * 0.02
    o = kernel(x, fc, fs, wq, wo)
    print(o.shape, o.dtype)


# revision 30
# speedup vs baseline: 1.0188x; 1.0176x over previous
"""GQA attention (B=2, S=2048, D=2048, H=32, G=8, hd=64) on 8 TRN2 cores.

Sharding: 2 batch groups x 4 TP ranks, NO collectives. Core c: batch
b=c//4, rank r=c%4. Each rank owns 2 KV groups (8 Q heads), computes a
PARTIAL output projection over its 512 local head-features, and the host
sums the 4 rank partials per batch.

Layout strategy (all transposes done on host):
  - x^T resident in SBUF; projections produce Q^T/K^T [feat, tok] and
    V [tok, feat] directly, so scores S^T [k, q] come out transpose-free
    and P^T blocks feed the PV matmul as the stationary operand.
  - scores: grp A on PE rows 0:64 / grp B on rows 64:128 via
    tile_position packing -> the two matmuls run concurrently.
  - softmax denominator: the PV stationary is [V_g (64) | ones (64)], so
    PSUM rows 64:128 accumulate sum_k(exp) replicated across 64
    partitions for free. Drain = copy denom to base-0 + fast approx
    reciprocal + fused multiply (no Ln/Exp table swaps, no broadcasts).
  - PSUM: one [128,2048] score tile holds TWO kb blocks (A|B|A|B), so
    EXP runs as one [128,2048] activation per kb-pair (less per-instr
    overhead); o accumulators single-buffered pair; remaining 2 banks
    shared by projections and the dripped output projection.
  - attention runs per 512-token group (qg); the output projection of
    group g is interleaved ("dripped") into group g+1's attention so it
    hides in the PE slack of the scalar-bound exp pipeline.
"""

import sys

sys.path.insert(0, "/opt/trn_rl_repo")

import numpy as np
import ml_dtypes

import concourse.bass as bass
import concourse.tile as tile
from concourse import bacc, mybir
from concourse.bass_utils import run_bass_kernel_spmd

BF16 = ml_dtypes.bfloat16
B, S, D = 2, 2048, 2048
H, G, HD = 32, 8, 64
N_CORES = 8
TP = 4
QF = 512   # q features per rank
DC = D // 128  # 16 dim chunks
QT = 1024  # q tokens per Q-projection tile

_CACHE = {}


def _build():
    f32 = mybir.dt.float32
    bf16 = mybir.dt.bfloat16
    nc = bacc.Bacc("TRN2", target_bir_lowering=False, debug=False, num_devices=N_CORES)

    xt = nc.dram_tensor("xt", [128, DC, S], bf16, kind="ExternalInput").ap()
    wqt = nc.dram_tensor("wqt", [128, DC, QF], bf16, kind="ExternalInput").ap()
    wkt = nc.dram_tensor("wkt", [128, DC, 128], bf16, kind="ExternalInput").ap()
    wvt = nc.dram_tensor("wvt", [128, DC, 128], bf16, kind="ExternalInput").ap()
    cosr = nc.dram_tensor("cosr", [128, S], bf16, kind="ExternalInput").ap()
    sinr = nc.dram_tensor("sinr", [128, S], bf16, kind="ExternalInput").ap()
    wot = nc.dram_tensor("wot", [128, 4, D], bf16, kind="ExternalInput").ap()
    out = nc.dram_tensor("out", [S, D], f32, kind="ExternalOutput").ap()

    Exp = mybir.ActivationFunctionType.Exp
    swap_mask = [i ^ 1 for i in range(32)]
    scale = float(1.0 / np.sqrt(HD))

    from contextlib import ExitStack
    with tile.TileContext(nc) as tc, ExitStack() as ctx:
        consts = ctx.enter_context(tc.tile_pool(name="consts", bufs=1))
        qk = ctx.enter_context(tc.tile_pool(name="qk", bufs=1))
        io = ctx.enter_context(tc.tile_pool(name="io", bufs=2))
        work = ctx.enter_context(tc.tile_pool(name="work", bufs=3))
        dr = ctx.enter_context(tc.tile_pool(name="dr", bufs=4))
        ost = ctx.enter_context(tc.tile_pool(name="ost", bufs=3))
        # PSUM (8 banks): sp 2x[128,1024]=4, o pairs bufs=3 x [128,512]=3,
        # pp 1 bank for dripped output-projection units
        psum = ctx.enter_context(tc.tile_pool(name="psum", bufs=2, space="PSUM"))
        opsum = ctx.enter_context(tc.tile_pool(name="opsum", bufs=3, space="PSUM"))
        pp = ctx.enter_context(tc.tile_pool(name="pp", bufs=1, space="PSUM"))

        # ---- load inputs (x^T split in 4 tiles so K-proj starts early)
        xt_sb = [
            qk.tile([128, 4, S], bf16, tag=f"xt{i}", name=f"xt_sb{i}")
            for i in range(4)
        ]
        wkt_sb = consts.tile([128, DC, 128], bf16, tag="wkt")
        nc.sync.dma_start(out=wkt_sb[:], in_=wkt[:])
        for i in range(4):
            nc.sync.dma_start(out=xt_sb[i][:], in_=xt[:, 4 * i:4 * i + 4, :])
        wvt_sb = consts.tile([128, DC, 128], bf16, tag="wvt")
        nc.sync.dma_start(out=wvt_sb[:], in_=wvt[:])
        cos_sb = consts.tile([128, S], bf16, tag="cos")
        nc.sync.dma_start(out=cos_sb[:], in_=cosr[:])
        sin_sb = consts.tile([128, S], bf16, tag="sin")
        nc.sync.dma_start(out=sin_sb[:], in_=sinr[:])
        wqt_sb = consts.tile([128, DC, QF], bf16, tag="wqt")
        nc.sync.dma_start(out=wqt_sb[:], in_=wqt[:])
        wot_sb = consts.tile([128, 4, D], bf16, tag="wot")
        nc.sync.dma_start(out=wot_sb[:], in_=wot[:])

        def xs(c):  # x^T chunk c
            return xt_sb[c // 4][:, c % 4, :]

        # ---- K^T projection, c-outer so (1) it starts once the first x^T
        # quarter lands and (2) the stationary is loaded once per c (4 nt
        # accumulate in parallel PSUM tiles)
        kt_sb = qk.tile([128, S], bf16, tag="kt")
        kps = [opsum.tile([128, 512], f32, tag="o", name=f"kps{i}")
               for i in range(3)]
        kps.append(pp.tile([128, 512], f32, tag="pp", name="kps3"))
        for c in range(DC):
            for nt in range(4):
                nc.tensor.matmul(
                    kps[nt], lhsT=wkt_sb[:, c, :],
                    rhs=xs(c)[:, nt * 512:(nt + 1) * 512],
                    start=(c == 0), stop=(c == DC - 1),
                )
        for nt in range(4):
            nc.vector.tensor_copy(kt_sb[:, nt * 512:(nt + 1) * 512], kps[nt])
        for hc in range(2):
            hsl = slice(hc * QT, (hc + 1) * QT)
            sw = io.tile([128, QT], bf16, tag="rsw")
            nc.vector.stream_shuffle(sw, kt_sb[:, hsl], swap_mask)
            nc.vector.tensor_mul(sw, sw, sin_sb[:, hsl])
            tmp = io.tile([128, QT], bf16, tag="rtmp")
            nc.vector.tensor_mul(tmp, kt_sb[:, hsl], cos_sb[:, hsl])
            nc.vector.tensor_add(kt_sb[:, hsl], sw, tmp)

        # ---- V projection into PV-stationary layout:
        # vtile[:, tb, 0:64]=V_A, 64:128=ones, 128:192=V_B, 192:256=ones
        vtile = qk.tile([128, DC, 256], bf16, tag="vtile")
        nc.vector.memset(vtile[:, :, 64:128], 1.0)
        nc.vector.memset(vtile[:, :, 192:256], 1.0)
        for tb in range(DC):
            ps = opsum.tile([128, 512], f32, tag="o")
            for c in range(DC):
                nc.tensor.matmul(
                    ps[:, 0:128],
                    lhsT=xs(c)[:, tb * 128:(tb + 1) * 128],
                    rhs=wvt_sb[:, c, :],
                    start=(c == 0), stop=(c == DC - 1),
                )
            nc.vector.tensor_copy(vtile[:, tb, 0:64], ps[:, 0:64])
            nc.vector.tensor_copy(vtile[:, tb, 128:192], ps[:, 64:128])

        # ---- Q^T projection (c-outer: stationary loaded once per (j,c),
        # both 512-token halves of the qtile accumulate in parallel) + RoPE
        qt_sb = [
            qk.tile([128, 4, QT], bf16, tag=f"qt{q}", name=f"qt_sb{q}")
            for q in range(2)
        ]
        ot_sb = [
            qk.tile([128, 4, QT], bf16, tag=f"ot{q}", name=f"ot_sb{q}")
            for q in range(2)
        ]
        def qproj_rope(q, j, pool_pair):
            """Project + RoPE one (qtile, j) Q tile."""
            if pool_pair:  # c-outer with two parallel accumulators
                qps = [opsum.tile([128, 512], f32, tag="o", name=f"qps{i}")
                       for i in range(2)]
                for c in range(DC):
                    for nt in range(2):
                        tsl = slice(q * QT + nt * 512,
                                    q * QT + (nt + 1) * 512)
                        nc.tensor.matmul(
                            qps[nt],
                            lhsT=wqt_sb[:, c, j * 128:(j + 1) * 128],
                            rhs=xs(c)[:, tsl],
                            start=(c == 0), stop=(c == DC - 1),
                        )
                for nt in range(2):
                    nc.vector.tensor_copy(
                        qt_sb[q][:, j, nt * 512:(nt + 1) * 512], qps[nt])
            else:  # hosted inside attention: single pp-slot accumulator
                for nt in range(2):
                    ps = pp.tile([128, 512], f32, tag="pp")
                    tsl = slice(q * QT + nt * 512, q * QT + (nt + 1) * 512)
                    for c in range(DC):
                        nc.tensor.matmul(
                            ps, lhsT=wqt_sb[:, c, j * 128:(j + 1) * 128],
                            rhs=xs(c)[:, tsl],
                            start=(c == 0), stop=(c == DC - 1),
                        )
                    nc.vector.tensor_copy(
                        qt_sb[q][:, j, nt * 512:(nt + 1) * 512], ps)
            qsl = slice(q * QT, (q + 1) * QT)
            t = qt_sb[q][:, j, :]
            sw = io.tile([128, QT], bf16, tag="rsw")
            nc.vector.stream_shuffle(sw, t, swap_mask)
            nc.vector.tensor_mul(sw, sw, sin_sb[:, qsl])
            tmp = io.tile([128, QT], bf16, tag="rtmp")
            nc.vector.tensor_mul(tmp, t, cos_sb[:, qsl])
            nc.vector.tensor_add(t, sw, tmp)

        # only (qtile0, j0) is projected up front; the other 7 Q tiles are
        # hosted at attention pass boundaries where the scalar-bound exp
        # pipeline leaves the PE idle
        qproj_rope(0, 0, pool_pair=True)
        hosted_q = [(0, 1), (0, 2), (0, 3), (1, 0), (1, 1), (1, 2), (1, 3)]

        # ---- attention per 512-token group + dripped output projection
        def oproj_unit(qg, tb, od, pool, use_scalar=False):
            """One output-projection unit: 4 matmuls + drain + DMA."""
            q = qg // 2
            gtb = qg * 4 + tb
            ps = pool.tile([128, 512], f32, tag="pp" if pool is pp else "o")
            for ic in range(4):
                nc.tensor.matmul(
                    ps,
                    lhsT=ot_sb[q][:, ic, (qg % 2) * 512 + tb * 128:
                                  (qg % 2) * 512 + (tb + 1) * 128],
                    rhs=wot_sb[:, ic, od * 512:(od + 1) * 512],
                    start=(ic == 0), stop=(ic == 3),
                )
            osb = ost.tile([128, 512], f32, tag="osb")
            if use_scalar:
                nc.scalar.activation(
                    osb, ps, mybir.ActivationFunctionType.Copy)
            else:
                nc.vector.tensor_copy(osb, ps)
            nc.sync.dma_start(
                out=out[gtb * 128:(gtb + 1) * 128, od * 512:(od + 1) * 512],
                in_=osb)

        pending = []
        for q in range(2):
            for qh in range(2):
                qg = 2 * q + qh
                qsl = slice(qh * 512, (qh + 1) * 512)
                for j in range(4):
                    oA = opsum.tile([128, 512], f32, tag="o", name="oA")
                    oB = opsum.tile([128, 512], f32, tag="o", name="oB")
                    for kb in range(DC):
                        ksl = slice(kb * 128, (kb + 1) * 128)
                        sp = psum.tile([128, 1024], f32, tag="sp")
                        nc.tensor.matmul(
                            sp[:, 0:512], lhsT=kt_sb[0:64, ksl],
                            rhs=qt_sb[q][0:64, j, qsl],
                            start=True, stop=True, tile_position=(0, 0),
                        )
                        nc.tensor.matmul(
                            sp[:, 512:1024], lhsT=kt_sb[64:128, ksl],
                            rhs=qt_sb[q][64:128, j, qsl],
                            start=True, stop=True, tile_position=(64, 0),
                        )
                        p = work.tile([128, 1024], bf16, tag="p")
                        nc.scalar.activation(p, sp, Exp, scale=scale)
                        nc.tensor.matmul(
                            oA, lhsT=vtile[:, kb, 0:128], rhs=p[:, 0:512],
                            start=(kb == 0), stop=(kb == DC - 1),
                        )
                        nc.tensor.matmul(
                            oB, lhsT=vtile[:, kb, 128:256], rhs=p[:, 512:1024],
                            start=(kb == 0), stop=(kb == DC - 1),
                        )
                        # drip one output-projection unit per 4 kb — fits in
                        # the PE slack of the scalar-bound exp pipeline
                        if pending and kb % 4 == 3:
                            pending.pop(0)(pp)
                    # drain: copy denom rows to base 0, fast reciprocal
                    # (base-aligned SBUF), multiply from PSUM data rows
                    for grp, o in ((0, oA), (1, oB)):
                        dcp = dr.tile([64, 512], f32, tag="dcp")
                        nc.vector.tensor_copy(dcp, o[64:128, :])
                        rec = dr.tile([64, 512], f32, tag="rec")
                        nc.vector.reciprocal_approx_fast(out=rec, in_=dcp)
                        nc.vector.tensor_mul(
                            ot_sb[q][64 * grp:64 * grp + 64, j, qsl],
                            o[0:64, :], rec)
                    # host one deferred Q projection in the pass boundary
                    if hosted_q:
                        hq, hj = hosted_q.pop(0)
                        qproj_rope(hq, hj, pool_pair=False)
                for tb in range(4):
                    for od in range(4):
                        pending.append(
                            (lambda pool, use_scalar=False, qg=qg, tb=tb,
                             od=od:
                             oproj_unit(qg, tb, od, pool, use_scalar)))
        # flush the remaining units, alternating pools so the drain copies
        # never serialize the accumulation
        i = 0
        while pending:
            pending.pop(0)(pp if i % 2 == 0 else opsum)
            i += 1

    nc.compile()
    return nc


def _prep_inputs(x, freqs_cos, freqs_sin, wqkv, wo):
    """Build per-core input maps (host-side shard + transpose + bf16 cast)."""
    ins = []
    wo_t = np.ascontiguousarray(wo.T)  # [in feat, out feat]
    cos_h = np.empty((128, S), np.float32)
    sin_h = np.empty((128, S), np.float32)
    cs = freqs_cos[:, 0, :]  # [S, 64]
    sn = freqs_sin[:, 0, :]
    for p in range(128):
        cos_h[p] = cs[:, p % 64]
        sin_h[p] = sn[:, p % 64] * (-1.0 if p % 2 == 0 else 1.0)
    cos_h = cos_h.astype(BF16)
    sin_h = sin_h.astype(BF16)

    for core in range(N_CORES):
        b, r = divmod(core, TP)
        xt_h = np.ascontiguousarray(
            x[b].T.reshape(DC, 128, S).transpose(1, 0, 2)).astype(BF16)
        # Q rows, permuted: j-tile j = [head 8r+j | head 8r+4+j]
        rows = []
        for j in range(4):
            for h in (8 * r + j, 8 * r + 4 + j):
                rows.extend(range(h * HD, (h + 1) * HD))
        wq_sel = wqkv[rows, :]  # [512, D]
        wqt_h = np.ascontiguousarray(
            wq_sel.T.reshape(DC, 128, QF).transpose(1, 0, 2)).astype(BF16)
        krows = []
        for g in (2 * r, 2 * r + 1):
            krows.extend(range(H * HD + g * HD, H * HD + (g + 1) * HD))
        wk_sel = wqkv[krows, :]  # [128, D]
        wkt_h = np.ascontiguousarray(
            wk_sel.T.reshape(DC, 128, 128).transpose(1, 0, 2)).astype(BF16)
        vrows = []
        for g in (2 * r, 2 * r + 1):
            vrows.extend(range((H + G) * HD + g * HD, (H + G) * HD + (g + 1) * HD))
        wv_sel = wqkv[vrows, :]  # [128, D]; cols 0:64=V_A feats, 64:128=V_B
        wvt_h = np.ascontiguousarray(
            wv_sel.T.reshape(DC, 128, 128).transpose(1, 0, 2)).astype(BF16)
        # wot: local head-feature rows, chunk ic=j: [head 8r+j | head 8r+4+j]
        perm = np.empty(4 * 128, np.int64)
        for j in range(4):
            for p in range(128):
                Hh = 8 * r + j + (4 if p >= 64 else 0)
                perm[j * 128 + p] = 64 * Hh + (p % 64)
        wot_h = np.ascontiguousarray(
            wo_t[perm, :].reshape(4, 128, D).transpose(1, 0, 2)).astype(BF16)
        ins.append({
            "xt": xt_h, "wqt": wqt_h, "wkt": wkt_h, "wvt": wvt_h,
            "cosr": cos_h, "sinr": sin_h, "wot": wot_h,
        })
    return ins


TRACE = False


def kernel(x, freqs_cos, freqs_sin, wqkv, wo):
    if "nc" not in _CACHE:
        _CACHE["nc"] = _build()
    nc = _CACHE["nc"]
    ins = _prep_inputs(x, freqs_cos, freqs_sin, wqkv, wo)
    res = run_bass_kernel_spmd(nc, ins, list(range(N_CORES)), trace=TRACE)
    _CACHE["res"] = res
    out = np.empty((B, S, D), np.float32)
    for b in range(B):
        acc = res.results[TP * b]["out"].astype(np.float32)
        for r in range(1, TP):
            acc = acc + res.results[TP * b + r]["out"]
        out[b] = acc
    return out


if __name__ == "__main__":
    rng = np.random.default_rng(0)
    x = rng.normal(size=(B, S, D)).astype(np.float32)
    fc = rng.random(size=(S, 1, HD)).astype(np.float32)
    fs = rng.random(size=(S, 1, HD)).astype(np.float32)
    wq = rng.normal(size=(3072, D)).astype(np.float32) * 0.02
    wo = rng.normal(size=(D, D)).astype(np.float32) * 0.02
    o = kernel(x, fc, fs, wq, wo)
    print(o.shape, o.dtype)


# revision 31
# speedup vs baseline: 1.0465x; 1.0272x over previous
"""GQA attention (B=2, S=2048, D=2048, H=32, G=8, hd=64) on 8 TRN2 cores.

Sharding: 2 batch groups x 4 TP ranks, NO collectives. Core c: batch
b=c//4, rank r=c%4. Each rank owns 2 KV groups (8 Q heads), computes a
PARTIAL output projection over its 512 local head-features, and the host
sums the 4 rank partials per batch.

Layout strategy (all transposes done on host):
  - x^T resident in SBUF; projections produce Q^T/K^T [feat, tok] and
    V [tok, feat] directly, so scores S^T [k, q] come out transpose-free
    and P^T blocks feed the PV matmul as the stationary operand.
  - scores: grp A on PE rows 0:64 / grp B on rows 64:128 via
    tile_position packing -> the two matmuls run concurrently.
  - softmax denominator: the PV stationary is [V_g (64) | ones (64)], so
    PSUM rows 64:128 accumulate sum_k(exp) replicated across 64
    partitions for free. Drain = copy denom to base-0 + fast approx
    reciprocal + fused multiply (no Ln/Exp table swaps, no broadcasts).
  - PSUM: one [128,2048] score tile holds TWO kb blocks (A|B|A|B), so
    EXP runs as one [128,2048] activation per kb-pair (less per-instr
    overhead); o accumulators single-buffered pair; remaining 2 banks
    shared by projections and the dripped output projection.
  - attention runs per 512-token group (qg); the output projection of
    group g is interleaved ("dripped") into group g+1's attention so it
    hides in the PE slack of the scalar-bound exp pipeline.
"""

import sys

sys.path.insert(0, "/opt/trn_rl_repo")

import numpy as np
import ml_dtypes

import concourse.bass as bass
import concourse.tile as tile
from concourse import bacc, mybir
from concourse.bass_utils import run_bass_kernel_spmd

BF16 = ml_dtypes.bfloat16
B, S, D = 2, 2048, 2048
H, G, HD = 32, 8, 64
N_CORES = 8
TP = 4
QF = 512   # q features per rank
DC = D // 128  # 16 dim chunks
QT = 1024  # q tokens per Q-projection tile

_CACHE = {}


def _build():
    f32 = mybir.dt.float32
    bf16 = mybir.dt.bfloat16
    nc = bacc.Bacc("TRN2", target_bir_lowering=False, debug=False, num_devices=N_CORES)

    xt = nc.dram_tensor("xt", [128, DC, S], bf16, kind="ExternalInput").ap()
    wqt = nc.dram_tensor("wqt", [128, DC, QF], bf16, kind="ExternalInput").ap()
    wkt = nc.dram_tensor("wkt", [128, DC, 128], bf16, kind="ExternalInput").ap()
    wvt = nc.dram_tensor("wvt", [128, DC, 128], bf16, kind="ExternalInput").ap()
    cosr = nc.dram_tensor("cosr", [128, S], bf16, kind="ExternalInput").ap()
    sinr = nc.dram_tensor("sinr", [128, S], bf16, kind="ExternalInput").ap()
    wot = nc.dram_tensor("wot", [128, 4, D], bf16, kind="ExternalInput").ap()
    out = nc.dram_tensor("out", [S, D], f32, kind="ExternalOutput").ap()

    Exp = mybir.ActivationFunctionType.Exp
    swap_mask = [i ^ 1 for i in range(32)]
    scale = float(1.0 / np.sqrt(HD))

    from contextlib import ExitStack
    with tile.TileContext(nc) as tc, ExitStack() as ctx:
        consts = ctx.enter_context(tc.tile_pool(name="consts", bufs=1))
        qk = ctx.enter_context(tc.tile_pool(name="qk", bufs=1))
        io = ctx.enter_context(tc.tile_pool(name="io", bufs=2))
        work = ctx.enter_context(tc.tile_pool(name="work", bufs=3))
        dr = ctx.enter_context(tc.tile_pool(name="dr", bufs=4))
        ost = ctx.enter_context(tc.tile_pool(name="ost", bufs=3))
        # PSUM (8 banks): sp 2x[128,1024]=4, o pairs bufs=3 x [128,512]=3,
        # pp 1 bank for dripped output-projection units
        psum = ctx.enter_context(tc.tile_pool(name="psum", bufs=2, space="PSUM"))
        opsum = ctx.enter_context(tc.tile_pool(name="opsum", bufs=3, space="PSUM"))
        pp = ctx.enter_context(tc.tile_pool(name="pp", bufs=1, space="PSUM"))

        # ---- load inputs (x^T split in 4 tiles so K-proj starts early)
        xt_sb = [
            qk.tile([128, 4, S], bf16, tag=f"xt{i}", name=f"xt_sb{i}")
            for i in range(4)
        ]
        wkt_sb = consts.tile([128, DC, 128], bf16, tag="wkt")
        nc.sync.dma_start(out=wkt_sb[:], in_=wkt[:])
        for i in range(4):
            nc.sync.dma_start(out=xt_sb[i][:], in_=xt[:, 4 * i:4 * i + 4, :])
        wvt_sb = consts.tile([128, DC, 128], bf16, tag="wvt")
        nc.sync.dma_start(out=wvt_sb[:], in_=wvt[:])
        cos_sb = consts.tile([128, S], bf16, tag="cos")
        nc.sync.dma_start(out=cos_sb[:], in_=cosr[:])
        sin_sb = consts.tile([128, S], bf16, tag="sin")
        nc.sync.dma_start(out=sin_sb[:], in_=sinr[:])
        wqt_sb = consts.tile([128, DC, QF], bf16, tag="wqt")
        nc.sync.dma_start(out=wqt_sb[:], in_=wqt[:])
        wot_sb = consts.tile([128, 4, D], bf16, tag="wot")
        nc.sync.dma_start(out=wot_sb[:], in_=wot[:])

        def xs(c):  # x^T chunk c
            return xt_sb[c // 4][:, c % 4, :]

        # ---- K^T projection, c-outer so (1) it starts once the first x^T
        # quarter lands and (2) the stationary is loaded once per c (4 nt
        # accumulate in parallel PSUM tiles)
        kt_sb = qk.tile([128, S], bf16, tag="kt")
        for hc in range(2):  # two nt-pair halves; RoPE(half0) overlaps half1
            kps = [opsum.tile([128, 512], f32, tag="o", name=f"kps{i}")
                   for i in range(2)]
            for c in range(DC):
                for nt in (2 * hc, 2 * hc + 1):
                    nc.tensor.matmul(
                        kps[nt - 2 * hc], lhsT=wkt_sb[:, c, :],
                        rhs=xs(c)[:, nt * 512:(nt + 1) * 512],
                        start=(c == 0), stop=(c == DC - 1),
                    )
            for nt in (2 * hc, 2 * hc + 1):
                nc.vector.tensor_copy(
                    kt_sb[:, nt * 512:(nt + 1) * 512], kps[nt - 2 * hc])
            hsl = slice(hc * QT, (hc + 1) * QT)
            sw = io.tile([128, QT], bf16, tag="rsw")
            nc.vector.stream_shuffle(sw, kt_sb[:, hsl], swap_mask)
            nc.vector.tensor_mul(sw, sw, sin_sb[:, hsl])
            tmp = io.tile([128, QT], bf16, tag="rtmp")
            nc.vector.tensor_mul(tmp, kt_sb[:, hsl], cos_sb[:, hsl])
            nc.vector.tensor_add(kt_sb[:, hsl], sw, tmp)

        # ---- V projection into PV-stationary layout:
        # vtile[:, tb, 0:64]=V_A, 64:128=ones, 128:192=V_B, 192:256=ones
        vtile = qk.tile([128, DC, 256], bf16, tag="vtile")
        nc.vector.memset(vtile[:, :, 64:128], 1.0)
        nc.vector.memset(vtile[:, :, 192:256], 1.0)
        for tb in range(DC):
            ps = opsum.tile([128, 512], f32, tag="o")
            for c in range(DC):
                nc.tensor.matmul(
                    ps[:, 0:128],
                    lhsT=xs(c)[:, tb * 128:(tb + 1) * 128],
                    rhs=wvt_sb[:, c, :],
                    start=(c == 0), stop=(c == DC - 1),
                )
            nc.vector.tensor_copy(vtile[:, tb, 0:64], ps[:, 0:64])
            nc.vector.tensor_copy(vtile[:, tb, 128:192], ps[:, 64:128])

        # ---- Q^T projection (c-outer: stationary loaded once per (j,c),
        # both 512-token halves of the qtile accumulate in parallel) + RoPE
        qt_sb = [
            qk.tile([128, 4, QT], bf16, tag=f"qt{q}", name=f"qt_sb{q}")
            for q in range(2)
        ]
        ot_sb = [
            qk.tile([128, 4, QT], bf16, tag=f"ot{q}", name=f"ot_sb{q}")
            for q in range(2)
        ]
        def qproj_rope(q, j, pool_pair):
            """Project + RoPE one (qtile, j) Q tile."""
            if pool_pair:  # c-outer with two parallel accumulators
                qps = [opsum.tile([128, 512], f32, tag="o", name=f"qps{i}")
                       for i in range(2)]
                for c in range(DC):
                    for nt in range(2):
                        tsl = slice(q * QT + nt * 512,
                                    q * QT + (nt + 1) * 512)
                        nc.tensor.matmul(
                            qps[nt],
                            lhsT=wqt_sb[:, c, j * 128:(j + 1) * 128],
                            rhs=xs(c)[:, tsl],
                            start=(c == 0), stop=(c == DC - 1),
                        )
                for nt in range(2):
                    nc.vector.tensor_copy(
                        qt_sb[q][:, j, nt * 512:(nt + 1) * 512], qps[nt])
            else:  # hosted inside attention: single pp-slot accumulator
                for nt in range(2):
                    ps = pp.tile([128, 512], f32, tag="pp")
                    tsl = slice(q * QT + nt * 512, q * QT + (nt + 1) * 512)
                    for c in range(DC):
                        nc.tensor.matmul(
                            ps, lhsT=wqt_sb[:, c, j * 128:(j + 1) * 128],
                            rhs=xs(c)[:, tsl],
                            start=(c == 0), stop=(c == DC - 1),
                        )
                    nc.vector.tensor_copy(
                        qt_sb[q][:, j, nt * 512:(nt + 1) * 512], ps)
            qsl = slice(q * QT, (q + 1) * QT)
            t = qt_sb[q][:, j, :]
            sw = io.tile([128, QT], bf16, tag="rsw")
            nc.vector.stream_shuffle(sw, t, swap_mask)
            nc.vector.tensor_mul(sw, sw, sin_sb[:, qsl])
            tmp = io.tile([128, QT], bf16, tag="rtmp")
            nc.vector.tensor_mul(tmp, t, cos_sb[:, qsl])
            nc.vector.tensor_add(t, sw, tmp)

        # only (qtile0, j0) is projected up front; the other 7 Q tiles are
        # hosted at attention pass boundaries where the scalar-bound exp
        # pipeline leaves the PE idle
        qproj_rope(0, 0, pool_pair=True)
        hosted_q = [(0, 1), (0, 2), (0, 3), (1, 0), (1, 1), (1, 2), (1, 3)]

        # ---- attention per 512-token group + dripped output projection
        def oproj_unit(qg, tb, od, pool, use_scalar=False):
            """One output-projection unit: 4 matmuls + drain + DMA."""
            q = qg // 2
            gtb = qg * 4 + tb
            ps = pool.tile([128, 512], f32, tag="pp" if pool is pp else "o")
            for ic in range(4):
                nc.tensor.matmul(
                    ps,
                    lhsT=ot_sb[q][:, ic, (qg % 2) * 512 + tb * 128:
                                  (qg % 2) * 512 + (tb + 1) * 128],
                    rhs=wot_sb[:, ic, od * 512:(od + 1) * 512],
                    start=(ic == 0), stop=(ic == 3),
                )
            osb = ost.tile([128, 512], f32, tag="osb")
            if use_scalar:
                nc.scalar.activation(
                    osb, ps, mybir.ActivationFunctionType.Copy)
            else:
                nc.vector.tensor_copy(osb, ps)
            nc.sync.dma_start(
                out=out[gtb * 128:(gtb + 1) * 128, od * 512:(od + 1) * 512],
                in_=osb)

        pending = []
        for q in range(2):
            for qh in range(2):
                qg = 2 * q + qh
                qsl = slice(qh * 512, (qh + 1) * 512)
                for j in range(4):
                    oA = opsum.tile([128, 512], f32, tag="o", name="oA")
                    oB = opsum.tile([128, 512], f32, tag="o", name="oB")
                    for kb in range(DC):
                        ksl = slice(kb * 128, (kb + 1) * 128)
                        sp = psum.tile([128, 1024], f32, tag="sp")
                        nc.tensor.matmul(
                            sp[:, 0:512], lhsT=kt_sb[0:64, ksl],
                            rhs=qt_sb[q][0:64, j, qsl],
                            start=True, stop=True, tile_position=(0, 0),
                        )
                        nc.tensor.matmul(
                            sp[:, 512:1024], lhsT=kt_sb[64:128, ksl],
                            rhs=qt_sb[q][64:128, j, qsl],
                            start=True, stop=True, tile_position=(64, 0),
                        )
                        p = work.tile([128, 1024], bf16, tag="p")
                        nc.scalar.activation(p, sp, Exp, scale=scale)
                        nc.tensor.matmul(
                            oA, lhsT=vtile[:, kb, 0:128], rhs=p[:, 0:512],
                            start=(kb == 0), stop=(kb == DC - 1),
                        )
                        nc.tensor.matmul(
                            oB, lhsT=vtile[:, kb, 128:256], rhs=p[:, 512:1024],
                            start=(kb == 0), stop=(kb == DC - 1),
                        )
                        # drip one output-projection unit per 4 kb — fits in
                        # the PE slack of the scalar-bound exp pipeline
                        if pending and kb % 4 == 3:
                            pending.pop(0)(pp)
                    # drain: copy denom rows to base 0, fast reciprocal
                    # (base-aligned SBUF), multiply from PSUM data rows
                    for grp, o in ((0, oA), (1, oB)):
                        dcp = dr.tile([64, 512], f32, tag="dcp")
                        nc.vector.tensor_copy(dcp, o[64:128, :])
                        rec = dr.tile([64, 512], f32, tag="rec")
                        nc.vector.reciprocal_approx_fast(out=rec, in_=dcp)
                        nc.vector.tensor_mul(
                            ot_sb[q][64 * grp:64 * grp + 64, j, qsl],
                            o[0:64, :], rec)
                    # host one deferred Q projection in the pass boundary
                    if hosted_q:
                        hq, hj = hosted_q.pop(0)
                        qproj_rope(hq, hj, pool_pair=False)
                for tb in range(4):
                    for od in range(4):
                        pending.append(
                            (lambda pool, use_scalar=False, qg=qg, tb=tb,
                             od=od:
                             oproj_unit(qg, tb, od, pool, use_scalar)))
        # flush the remaining units, alternating pools so the drain copies
        # never serialize the accumulation
        i = 0
        while pending:
            pending.pop(0)(pp if i % 2 == 0 else opsum)
            i += 1

    nc.compile()
    return nc


def _prep_inputs(x, freqs_cos, freqs_sin, wqkv, wo):
    """Build per-core input maps (host-side shard + transpose + bf16 cast)."""
    ins = []
    wo_t = np.ascontiguousarray(wo.T)  # [in feat, out feat]
    cos_h = np.empty((128, S), np.float32)
    sin_h = np.empty((128, S), np.float32)
    cs = freqs_cos[:, 0, :]  # [S, 64]
    sn = freqs_sin[:, 0, :]
    for p in range(128):
        cos_h[p] = cs[:, p % 64]
        sin_h[p] = sn[:, p % 64] * (-1.0 if p % 2 == 0 else 1.0)
    cos_h = cos_h.astype(BF16)
    sin_h = sin_h.astype(BF16)

    for core in range(N_CORES):
        b, r = divmod(core, TP)
        xt_h = np.ascontiguousarray(
            x[b].T.reshape(DC, 128, S).transpose(1, 0, 2)).astype(BF16)
        # Q rows, permuted: j-tile j = [head 8r+j | head 8r+4+j]
        rows = []
        for j in range(4):
            for h in (8 * r + j, 8 * r + 4 + j):
                rows.extend(range(h * HD, (h + 1) * HD))
        wq_sel = wqkv[rows, :]  # [512, D]
        wqt_h = np.ascontiguousarray(
            wq_sel.T.reshape(DC, 128, QF).transpose(1, 0, 2)).astype(BF16)
        krows = []
        for g in (2 * r, 2 * r + 1):
            krows.extend(range(H * HD + g * HD, H * HD + (g + 1) * HD))
        wk_sel = wqkv[krows, :]  # [128, D]
        wkt_h = np.ascontiguousarray(
            wk_sel.T.reshape(DC, 128, 128).transpose(1, 0, 2)).astype(BF16)
        vrows = []
        for g in (2 * r, 2 * r + 1):
            vrows.extend(range((H + G) * HD + g * HD, (H + G) * HD + (g + 1) * HD))
        wv_sel = wqkv[vrows, :]  # [128, D]; cols 0:64=V_A feats, 64:128=V_B
        wvt_h = np.ascontiguousarray(
            wv_sel.T.reshape(DC, 128, 128).transpose(1, 0, 2)).astype(BF16)
        # wot: local head-feature rows, chunk ic=j: [head 8r+j | head 8r+4+j]
        perm = np.empty(4 * 128, np.int64)
        for j in range(4):
            for p in range(128):
                Hh = 8 * r + j + (4 if p >= 64 else 0)
                perm[j * 128 + p] = 64 * Hh + (p % 64)
        wot_h = np.ascontiguousarray(
            wo_t[perm, :].reshape(4, 128, D).transpose(1, 0, 2)).astype(BF16)
        ins.append({
            "xt": xt_h, "wqt": wqt_h, "wkt": wkt_h, "wvt": wvt_h,
            "cosr": cos_h, "sinr": sin_h, "wot": wot_h,
        })
    return ins


TRACE = False


def kernel(x, freqs_cos, freqs_sin, wqkv, wo):
    if "nc" not in _CACHE:
        _CACHE["nc"] = _build()
    nc = _CACHE["nc"]
    ins = _prep_inputs(x, freqs_cos, freqs_sin, wqkv, wo)
    res = run_bass_kernel_spmd(nc, ins, list(range(N_CORES)), trace=TRACE)
    _CACHE["res"] = res
    out = np.empty((B, S, D), np.float32)
    for b in range(B):
        acc = res.results[TP * b]["out"].astype(np.float32)
        for r in range(1, TP):
            acc = acc + res.results[TP * b + r]["out"]
        out[b] = acc
    return out


if __name__ == "__main__":
    rng = np.random.default_rng(0)
    x = rng.normal(size=(B, S, D)).astype(np.float32)
    fc = rng.random(size=(S, 1, HD)).astype(np.float32)
    fs = rng.random(size=(S, 1, HD)).astype(np.float32)
    wq = rng.normal(size=(3072, D)).astype(np.float32) * 0.02
    wo = rng.normal(size=(D, D)).astype(np.float32) * 0.02
    o = kernel(x, fc, fs, wq, wo)
    print(o.shape, o.dtype)


# revision 34
# speedup vs baseline: 1.0489x; 1.0023x over previous
"""GQA attention (B=2, S=2048, D=2048, H=32, G=8, hd=64) on 8 TRN2 cores.

Sharding: 2 batch groups x 4 TP ranks, NO collectives. Core c: batch
b=c//4, rank r=c%4. Each rank owns 2 KV groups (8 Q heads), computes a
PARTIAL output projection over its 512 local head-features, and the host
sums the 4 rank partials per batch.

Layout strategy (all transposes done on host):
  - x^T resident in SBUF; projections produce Q^T/K^T [feat, tok] and
    V [tok, feat] directly, so scores S^T [k, q] come out transpose-free
    and P^T blocks feed the PV matmul as the stationary operand.
  - scores: grp A on PE rows 0:64 / grp B on rows 64:128 via
    tile_position packing -> the two matmuls run concurrently.
  - softmax denominator: the PV stationary is [V_g (64) | ones (64)], so
    PSUM rows 64:128 accumulate sum_k(exp) replicated across 64
    partitions for free. Drain = copy denom to base-0 + fast approx
    reciprocal + fused multiply (no Ln/Exp table swaps, no broadcasts).
  - PSUM: one [128,2048] score tile holds TWO kb blocks (A|B|A|B), so
    EXP runs as one [128,2048] activation per kb-pair (less per-instr
    overhead); o accumulators single-buffered pair; remaining 2 banks
    shared by projections and the dripped output projection.
  - attention runs per 512-token group (qg); the output projection of
    group g is interleaved ("dripped") into group g+1's attention so it
    hides in the PE slack of the scalar-bound exp pipeline.
"""

import sys

sys.path.insert(0, "/opt/trn_rl_repo")

import numpy as np
import ml_dtypes

import concourse.bass as bass
import concourse.tile as tile
from concourse import bacc, mybir
from concourse.bass_utils import run_bass_kernel_spmd

BF16 = ml_dtypes.bfloat16
B, S, D = 2, 2048, 2048
H, G, HD = 32, 8, 64
N_CORES = 8
TP = 4
QF = 512   # q features per rank
DC = D // 128  # 16 dim chunks
QT = 1024  # q tokens per Q-projection tile

_CACHE = {}


def _build():
    f32 = mybir.dt.float32
    bf16 = mybir.dt.bfloat16
    nc = bacc.Bacc("TRN2", target_bir_lowering=False, debug=False, num_devices=N_CORES)

    xt = nc.dram_tensor("xt", [128, DC, S], bf16, kind="ExternalInput").ap()
    wqt = nc.dram_tensor("wqt", [128, DC, QF], bf16, kind="ExternalInput").ap()
    wkt = nc.dram_tensor("wkt", [128, DC, 128], bf16, kind="ExternalInput").ap()
    wvt = nc.dram_tensor("wvt", [128, DC, 128], bf16, kind="ExternalInput").ap()
    cosr = nc.dram_tensor("cosr", [128, S], bf16, kind="ExternalInput").ap()
    sinr = nc.dram_tensor("sinr", [128, S], bf16, kind="ExternalInput").ap()
    wot = nc.dram_tensor("wot", [128, 4, D], bf16, kind="ExternalInput").ap()
    out = nc.dram_tensor("out", [S, D], f32, kind="ExternalOutput").ap()

    Exp = mybir.ActivationFunctionType.Exp
    swap_mask = [i ^ 1 for i in range(32)]
    scale = float(1.0 / np.sqrt(HD))

    from contextlib import ExitStack
    with tile.TileContext(nc) as tc, ExitStack() as ctx:
        consts = ctx.enter_context(tc.tile_pool(name="consts", bufs=1))
        qk = ctx.enter_context(tc.tile_pool(name="qk", bufs=1))
        io = ctx.enter_context(tc.tile_pool(name="io", bufs=2))
        work = ctx.enter_context(tc.tile_pool(name="work", bufs=3))
        dr = ctx.enter_context(tc.tile_pool(name="dr", bufs=4))
        ost = ctx.enter_context(tc.tile_pool(name="ost", bufs=4))
        # PSUM (8 banks): sp 2x[128,1024]=4, o pairs bufs=3 x [128,512]=3,
        # pp 1 bank for dripped output-projection units
        psum = ctx.enter_context(tc.tile_pool(name="psum", bufs=2, space="PSUM"))
        opsum = ctx.enter_context(tc.tile_pool(name="opsum", bufs=3, space="PSUM"))
        pp = ctx.enter_context(tc.tile_pool(name="pp", bufs=1, space="PSUM"))

        # ---- load inputs (x^T split in 4 tiles so K-proj starts early)
        xt_sb = [
            qk.tile([128, 4, S], bf16, tag=f"xt{i}", name=f"xt_sb{i}")
            for i in range(4)
        ]
        wkt_sb = consts.tile([128, DC, 128], bf16, tag="wkt")
        nc.sync.dma_start(out=wkt_sb[:], in_=wkt[:])
        for i in range(4):
            nc.sync.dma_start(out=xt_sb[i][:], in_=xt[:, 4 * i:4 * i + 4, :])
        wvt_sb = consts.tile([128, DC, 128], bf16, tag="wvt")
        nc.sync.dma_start(out=wvt_sb[:], in_=wvt[:])
        cos_sb = consts.tile([128, S], bf16, tag="cos")
        nc.sync.dma_start(out=cos_sb[:], in_=cosr[:])
        sin_sb = consts.tile([128, S], bf16, tag="sin")
        nc.sync.dma_start(out=sin_sb[:], in_=sinr[:])
        wqt_sb = consts.tile([128, DC, QF], bf16, tag="wqt")
        nc.sync.dma_start(out=wqt_sb[:], in_=wqt[:])
        wot_sb = consts.tile([128, 4, D], bf16, tag="wot")
        nc.sync.dma_start(out=wot_sb[:], in_=wot[:])

        def xs(c):  # x^T chunk c
            return xt_sb[c // 4][:, c % 4, :]

        # ---- K^T projection, c-outer so (1) it starts once the first x^T
        # quarter lands and (2) the stationary is loaded once per c (4 nt
        # accumulate in parallel PSUM tiles)
        kt_sb = qk.tile([128, S], bf16, tag="kt")
        for hc in range(2):  # two nt-pair halves; RoPE(half0) overlaps half1
            kps = [opsum.tile([128, 512], f32, tag="o", name=f"kps{i}")
                   for i in range(2)]
            for c in range(DC):
                for nt in (2 * hc, 2 * hc + 1):
                    nc.tensor.matmul(
                        kps[nt - 2 * hc], lhsT=wkt_sb[:, c, :],
                        rhs=xs(c)[:, nt * 512:(nt + 1) * 512],
                        start=(c == 0), stop=(c == DC - 1),
                    )
            for nt in (2 * hc, 2 * hc + 1):
                nc.vector.tensor_copy(
                    kt_sb[:, nt * 512:(nt + 1) * 512], kps[nt - 2 * hc])
            hsl = slice(hc * QT, (hc + 1) * QT)
            sw = io.tile([128, QT], bf16, tag="rsw")
            nc.vector.stream_shuffle(sw, kt_sb[:, hsl], swap_mask)
            nc.vector.tensor_mul(sw, sw, sin_sb[:, hsl])
            tmp = io.tile([128, QT], bf16, tag="rtmp")
            nc.vector.tensor_mul(tmp, kt_sb[:, hsl], cos_sb[:, hsl])
            nc.vector.tensor_add(kt_sb[:, hsl], sw, tmp)

        # ---- V projection into PV-stationary layout:
        # vtile[:, tb, 0:64]=V_A, 64:128=ones, 128:192=V_B, 192:256=ones
        vtile = qk.tile([128, DC, 256], bf16, tag="vtile")
        nc.vector.memset(vtile[:, :, 64:128], 1.0)
        nc.vector.memset(vtile[:, :, 192:256], 1.0)
        for tb in range(DC):
            ps = opsum.tile([128, 512], f32, tag="o")
            for c in range(DC):
                nc.tensor.matmul(
                    ps[:, 0:128],
                    lhsT=xs(c)[:, tb * 128:(tb + 1) * 128],
                    rhs=wvt_sb[:, c, :],
                    start=(c == 0), stop=(c == DC - 1),
                )
            nc.vector.tensor_copy(vtile[:, tb, 0:64], ps[:, 0:64])
            nc.vector.tensor_copy(vtile[:, tb, 128:192], ps[:, 64:128])

        # ---- Q^T projection (c-outer: stationary loaded once per (j,c),
        # both 512-token halves of the qtile accumulate in parallel) + RoPE
        qt_sb = [
            qk.tile([128, 4, QT], bf16, tag=f"qt{q}", name=f"qt_sb{q}")
            for q in range(2)
        ]
        ot_sb = [
            qk.tile([128, 4, QT], bf16, tag=f"ot{q}", name=f"ot_sb{q}")
            for q in range(2)
        ]
        def qproj_rope(q, j, pool_pair):
            """Project + RoPE one (qtile, j) Q tile."""
            if pool_pair:  # c-outer with two parallel accumulators
                qps = [opsum.tile([128, 512], f32, tag="o", name=f"qps{i}")
                       for i in range(2)]
                for c in range(DC):
                    for nt in range(2):
                        tsl = slice(q * QT + nt * 512,
                                    q * QT + (nt + 1) * 512)
                        nc.tensor.matmul(
                            qps[nt],
                            lhsT=wqt_sb[:, c, j * 128:(j + 1) * 128],
                            rhs=xs(c)[:, tsl],
                            start=(c == 0), stop=(c == DC - 1),
                        )
                for nt in range(2):
                    nc.vector.tensor_copy(
                        qt_sb[q][:, j, nt * 512:(nt + 1) * 512], qps[nt])
            else:  # hosted inside attention: single pp-slot accumulator
                for nt in range(2):
                    ps = pp.tile([128, 512], f32, tag="pp")
                    tsl = slice(q * QT + nt * 512, q * QT + (nt + 1) * 512)
                    for c in range(DC):
                        nc.tensor.matmul(
                            ps, lhsT=wqt_sb[:, c, j * 128:(j + 1) * 128],
                            rhs=xs(c)[:, tsl],
                            start=(c == 0), stop=(c == DC - 1),
                        )
                    nc.vector.tensor_copy(
                        qt_sb[q][:, j, nt * 512:(nt + 1) * 512], ps)
            qsl = slice(q * QT, (q + 1) * QT)
            t = qt_sb[q][:, j, :]
            sw = io.tile([128, QT], bf16, tag="rsw")
            nc.vector.stream_shuffle(sw, t, swap_mask)
            nc.vector.tensor_mul(sw, sw, sin_sb[:, qsl])
            tmp = io.tile([128, QT], bf16, tag="rtmp")
            nc.vector.tensor_mul(tmp, t, cos_sb[:, qsl])
            nc.vector.tensor_add(t, sw, tmp)

        # only (qtile0, j0) is projected up front; the other 7 Q tiles are
        # hosted at attention pass boundaries where the scalar-bound exp
        # pipeline leaves the PE idle
        qproj_rope(0, 0, pool_pair=True)
        hosted_q = [(0, 1), (0, 2), (0, 3), (1, 0), (1, 1), (1, 2), (1, 3)]

        # ---- attention per 512-token group + dripped output projection
        def oproj_unit(qg, tb, od, pool, use_scalar=False):
            """One output-projection unit: 4 matmuls + drain + DMA."""
            q = qg // 2
            gtb = qg * 4 + tb
            ps = pool.tile([128, 512], f32, tag="pp" if pool is pp else "o")
            for ic in range(4):
                nc.tensor.matmul(
                    ps,
                    lhsT=ot_sb[q][:, ic, (qg % 2) * 512 + tb * 128:
                                  (qg % 2) * 512 + (tb + 1) * 128],
                    rhs=wot_sb[:, ic, od * 512:(od + 1) * 512],
                    start=(ic == 0), stop=(ic == 3),
                )
            osb = ost.tile([128, 512], f32, tag="osb")
            if use_scalar:
                nc.scalar.activation(
                    osb, ps, mybir.ActivationFunctionType.Copy)
            else:
                nc.vector.tensor_copy(osb, ps)
            nc.sync.dma_start(
                out=out[gtb * 128:(gtb + 1) * 128, od * 512:(od + 1) * 512],
                in_=osb)

        pending = []
        for q in range(2):
            for qh in range(2):
                qg = 2 * q + qh
                qsl = slice(qh * 512, (qh + 1) * 512)
                for j in range(4):
                    oA = opsum.tile([128, 512], f32, tag="o", name="oA")
                    oB = opsum.tile([128, 512], f32, tag="o", name="oB")
                    for kb in range(DC):
                        ksl = slice(kb * 128, (kb + 1) * 128)
                        sp = psum.tile([128, 1024], f32, tag="sp")
                        nc.tensor.matmul(
                            sp[:, 0:512], lhsT=kt_sb[0:64, ksl],
                            rhs=qt_sb[q][0:64, j, qsl],
                            start=True, stop=True, tile_position=(0, 0),
                        )
                        nc.tensor.matmul(
                            sp[:, 512:1024], lhsT=kt_sb[64:128, ksl],
                            rhs=qt_sb[q][64:128, j, qsl],
                            start=True, stop=True, tile_position=(64, 0),
                        )
                        p = work.tile([128, 1024], bf16, tag="p")
                        nc.scalar.activation(p, sp, Exp, scale=scale)
                        nc.tensor.matmul(
                            oA, lhsT=vtile[:, kb, 0:128], rhs=p[:, 0:512],
                            start=(kb == 0), stop=(kb == DC - 1),
                        )
                        nc.tensor.matmul(
                            oB, lhsT=vtile[:, kb, 128:256], rhs=p[:, 512:1024],
                            start=(kb == 0), stop=(kb == DC - 1),
                        )
                        # drip one output-projection unit per 4 kb — fits in
                        # the PE slack of the scalar-bound exp pipeline
                        if pending and kb % 4 == 3:
                            pending.pop(0)(pp)
                    # drain: copy denom rows to base 0, fast reciprocal
                    # (base-aligned SBUF), multiply from PSUM data rows
                    for grp, o in ((0, oA), (1, oB)):
                        dcp = dr.tile([64, 512], f32, tag="dcp")
                        nc.vector.tensor_copy(dcp, o[64:128, :])
                        rec = dr.tile([64, 512], f32, tag="rec")
                        nc.vector.reciprocal_approx_fast(out=rec, in_=dcp)
                        nc.vector.tensor_mul(
                            ot_sb[q][64 * grp:64 * grp + 64, j, qsl],
                            o[0:64, :], rec)
                    # host deferred Q projections in the pass boundary; two
                    # at the first boundary so every tile has one full pass
                    # of lead time before the pass that reads it
                    for _ in range(2 if (qg == 0 and j == 0) else 1):
                        if hosted_q:
                            hq, hj = hosted_q.pop(0)
                            qproj_rope(hq, hj, pool_pair=False)
                for tb in range(4):
                    for od in range(4):
                        pending.append(
                            (lambda pool, use_scalar=False, qg=qg, tb=tb,
                             od=od:
                             oproj_unit(qg, tb, od, pool, use_scalar)))
        # flush the remaining units, alternating pools so the drain copies
        # never serialize the accumulation; scalar is idle at the tail, so
        # it takes half the drain copies
        i = 0
        while pending:
            fn = pending.pop(0)
            fn(pp if i % 2 == 0 else opsum, use_scalar=(i % 2 == 1))
            i += 1

    nc.compile()
    return nc


def _prep_inputs(x, freqs_cos, freqs_sin, wqkv, wo):
    """Build per-core input maps (host-side shard + transpose + bf16 cast)."""
    ins = []
    wo_t = np.ascontiguousarray(wo.T)  # [in feat, out feat]
    cos_h = np.empty((128, S), np.float32)
    sin_h = np.empty((128, S), np.float32)
    cs = freqs_cos[:, 0, :]  # [S, 64]
    sn = freqs_sin[:, 0, :]
    for p in range(128):
        cos_h[p] = cs[:, p % 64]
        sin_h[p] = sn[:, p % 64] * (-1.0 if p % 2 == 0 else 1.0)
    cos_h = cos_h.astype(BF16)
    sin_h = sin_h.astype(BF16)

    for core in range(N_CORES):
        b, r = divmod(core, TP)
        xt_h = np.ascontiguousarray(
            x[b].T.reshape(DC, 128, S).transpose(1, 0, 2)).astype(BF16)
        # Q rows, permuted: j-tile j = [head 8r+j | head 8r+4+j]
        rows = []
        for j in range(4):
            for h in (8 * r + j, 8 * r + 4 + j):
                rows.extend(range(h * HD, (h + 1) * HD))
        wq_sel = wqkv[rows, :]  # [512, D]
        wqt_h = np.ascontiguousarray(
            wq_sel.T.reshape(DC, 128, QF).transpose(1, 0, 2)).astype(BF16)
        krows = []
        for g in (2 * r, 2 * r + 1):
            krows.extend(range(H * HD + g * HD, H * HD + (g + 1) * HD))
        wk_sel = wqkv[krows, :]  # [128, D]
        wkt_h = np.ascontiguousarray(
            wk_sel.T.reshape(DC, 128, 128).transpose(1, 0, 2)).astype(BF16)
        vrows = []
        for g in (2 * r, 2 * r + 1):
            vrows.extend(range((H + G) * HD + g * HD, (H + G) * HD + (g + 1) * HD))
        wv_sel = wqkv[vrows, :]  # [128, D]; cols 0:64=V_A feats, 64:128=V_B
        wvt_h = np.ascontiguousarray(
            wv_sel.T.reshape(DC, 128, 128).transpose(1, 0, 2)).astype(BF16)
        # wot: local head-feature rows, chunk ic=j: [head 8r+j | head 8r+4+j]
        perm = np.empty(4 * 128, np.int64)
        for j in range(4):
            for p in range(128):
                Hh = 8 * r + j + (4 if p >= 64 else 0)
                perm[j * 128 + p] = 64 * Hh + (p % 64)
        wot_h = np.ascontiguousarray(
            wo_t[perm, :].reshape(4, 128, D).transpose(1, 0, 2)).astype(BF16)
        ins.append({
            "xt": xt_h, "wqt": wqt_h, "wkt": wkt_h, "wvt": wvt_h,
            "cosr": cos_h, "sinr": sin_h, "wot": wot_h,
        })
    return ins


TRACE = False


def kernel(x, freqs_cos, freqs_sin, wqkv, wo):
    if "nc" not in _CACHE:
        _CACHE["nc"] = _build()
    nc = _CACHE["nc"]
    ins = _prep_inputs(x, freqs_cos, freqs_sin, wqkv, wo)
    res = run_bass_kernel_spmd(nc, ins, list(range(N_CORES)), trace=TRACE)
    _CACHE["res"] = res
    out = np.empty((B, S, D), np.float32)
    for b in range(B):
        acc = res.results[TP * b]["out"].astype(np.float32)
        for r in range(1, TP):
            acc = acc + res.results[TP * b + r]["out"]
        out[b] = acc
    return out


if __name__ == "__main__":
    rng = np.random.default_rng(0)
    x = rng.normal(size=(B, S, D)).astype(np.float32)
    fc = rng.random(size=(S, 1, HD)).astype(np.float32)
    fs = rng.random(size=(S, 1, HD)).astype(np.float32)
    wq = rng.normal(size=(3072, D)).astype(np.float32) * 0.02
    wo = rng.normal(size=(D, D)).astype(np.float32) * 0.02
    o = kernel(x, fc, fs, wq, wo)
    print(o.shape, o.dtype)
